# revision 1
# baseline (speedup 1.0000x reference)
"""Trainium2 Bass kernel for nn_CoTLayer (LN -> MHA w/ causal-repeat mask -> residual -> LN -> FFN).

Sharding (8 cores):
  - Attention is head-parallel: core c owns heads 2c, 2c+1. Each core computes
    Q/K/V projections for its heads over all tokens (Q from LN1'd query, K/V
    from raw context), masked softmax, and attn output oT (128 hd-dims x 2048 tokens).
  - One AllToAll redistributes oT from head-sharding to token-sharding.
  - Out-proj, residual, LN2 and FFN are token-parallel: core c owns tokens
    [256c, 256c+256) and uses full wo/w1/w2.
  - Host does input transposes/casts (activations are feature-major on device)
    and the final gather/transpose.

All matmuls run in bf16 (fp32 PSUM accumulation). LayerNorms are computed
feature-major: column sums via ones-matmuls on the PE, normalization folded
into the projection (LN1) or applied via broadcast rows (LN2). Row->128-
partition broadcasts use K=1 fp32 matmuls (ones stationary) instead of DRAM
round-trips. Softmax uses exp(scale*s - C) without max-subtraction; the
causal-repeat mask is applied by restricting each 128-key tile's score/exp/AV
work to the valid query suffix and zeroing the 128-wide diagonal triangle on
the probabilities with a 0/1 mask multiply (DVE). Denominators come from an
appended ones-column on V; their reciprocals run on the ACT engine.
K/V projections for both batches are emitted before the LN1-dependent q-proj
so the PE stays busy during the LN1 stats chain.
"""
import os
import sys
import numpy as np

B, S, R, D, H, FF = 2, 1024, 4, 1024, 16, 4096
HD = D // H              # 64
NCORES = 8
HPC = H // NCORES        # 2
T = B * S                # 2048
TC = B * S * R           # 8192
TSL = T // NCORES        # 256
SB = TC // B             # 4096 context tokens per batch
EXP_C = 16.0
SCALE = float(1.0 / np.sqrt(HD))
EPS = 1e-5

_CACHE = {}


def _import_concourse():
    for p in ("/opt/trn_rl_repo", "/root/.axon_site/_ro/trn_rl_repo"):
        if os.path.isdir(p) and p not in sys.path:
            sys.path.insert(0, p)
    import concourse.bass as bass            # noqa
    import concourse.tile as tile            # noqa
    from concourse import mybir              # noqa
    from concourse.bass_utils import run_bass_kernel_spmd  # noqa
    return bass, tile, mybir, run_bass_kernel_spmd


def _bcast_ap(bass, src_ap, nparts):
    """AP reading a (1, N) DRAM row broadcast to nparts partitions."""
    return bass.AP(tensor=src_ap.tensor, offset=src_ap.offset,
                   ap=[[0, nparts]] + list(src_ap.ap[1:]))


def _row_ap(bass, src_ap, n):
    """AP viewing a contiguous DRAM tile as a single (1, n) row."""
    return bass.AP(tensor=src_ap.tensor, offset=src_ap.offset,
                   ap=[[0, 1], [1, n]])


def _build_program(flags):
    """Build the SPMD Bass program (identical on all cores; per-core data via inputs)."""
    bass, tile, mybir, _ = _import_concourse()
    from contextlib import ExitStack

    f32 = mybir.dt.float32
    bf16 = mybir.dt.bfloat16
    AF = mybir.ActivationFunctionType
    ALU = mybir.AluOpType
    has_rq, has_bk, has_bv, has_bo, has_b2 = flags

    nc = bass.Bass()
    dp = nc.declare_dram_parameter
    qT_d = dp("qT", [D, T], bf16, isOutput=False)
    cT_d = dp("cT", [D, TC], bf16, isOutput=False)
    qsT_d = dp("qsT", [D, TSL], f32, isOutput=False)
    wq_d = dp("wq", [128, 8, 128], bf16, isOutput=False)     # [p, ks, hd']
    nu_d = dp("nu", [1, 128], bf16, isOutput=False)          # -colsums(wq_eff_c)
    wk_d = dp("wk", [128, 8, 128], bf16, isOutput=False)
    wv_d = dp("wv", [128, 8, 128], bf16, isOutput=False)
    wo_d = dp("wo", [128, 8, 1024], bf16, isOutput=False)
    w1_d = dp("w1", [128, 8, 4096], bf16, isOutput=False)
    w2_d = dp("w2", [128, 32, 1024], bf16, isOutput=False)
    bffT_d = dp("bffT", [128, 32], f32, isOutput=False)      # gelu bias per ff-dim
    tri_d = dp("tri", [128, 128], bf16, isOutput=False)      # upper-tri 0/1
    ones_d = dp("ones_bf", [128, 1], bf16, isOutput=False)
    onesf_d = dp("ones_f32", [1, 128], f32, isOutput=False)
    if has_rq:
        rq_d = dp("rqT", [128, 1], f32, isOutput=False)
    if has_bk:
        bk_d = dp("bkT", [128, 1], f32, isOutput=False)
    if has_bv:
        onesrow_d = dp("ones_row", [1, 128], bf16, isOutput=False)
        bvr_d = dp("bvr", [1, 128], bf16, isOutput=False)
    if has_bo:
        bo_d = dp("boT", [128, 8], f32, isOutput=False)
    if has_b2:
        b2_d = dp("b2T", [128, 8], f32, isOutput=False)
    outT_d = dp("outT", [D, TSL], f32, isOutput=True)
    debug = bool(os.environ.get("KERNEL_DEBUG"))
    if debug:
        dbgx_d = dp("dbg_x", [D, TSL], f32, isOutput=True)
        dbgh_d = dp("dbg_h", [D, TSL], f32, isOutput=True)
        dbgo_d = dp("dbg_o", [D, TSL], f32, isOutput=True)
        dbgf_d = dp("dbg_f", [FF, TSL], f32, isOutput=True)

    with ExitStack() as top:
        tc = top.enter_context(tile.TileContext(nc))
        const = top.enter_context(tc.tile_pool(name="const", bufs=1))
        # PSUM pools: sc(2 banks)x2 + proj(1 bank)x2 + o(1 bank)x2 = 8 banks
        psum_sc = top.enter_context(tc.tile_pool(name="psc", bufs=2, space="PSUM"))
        psum_pj = top.enter_context(tc.tile_pool(name="ppj", bufs=2, space="PSUM"))
        psum_o = top.enter_context(tc.tile_pool(name="po", bufs=2, space="PSUM"))
        dram = top.enter_context(tc.tile_pool(name="dram", bufs=1, space="DRAM"))
        wpool = top.enter_context(tc.tile_pool(name="wpool", bufs=1))

        # ---- persistent constants / small weights ----
        negc = const.tile([128, 1], f32, tag="negc")
        nc.vector.memset(negc[:], -EXP_C)
        ones_col = const.tile([128, 1], bf16, tag="ones")
        nc.sync.dma_start(ones_col[:], ones_d[:])
        onesf = const.tile([1, 128], f32, tag="onesf")
        nc.sync.dma_start(onesf[:], onesf_d[:])
        tri = const.tile([128, 128], bf16, tag="tri")
        nc.sync.dma_start(tri[:], tri_d[:])
        wq_sb = const.tile([128, 8, 128], bf16, tag="wq")
        nc.sync.dma_start(wq_sb[:], wq_d[:])
        wk_sb = const.tile([128, 8, 128], bf16, tag="wk")
        nc.sync.dma_start(wk_sb[:], wk_d[:])
        wv_sb = const.tile([128, 8, 128], bf16, tag="wv")
        nc.sync.dma_start(wv_sb[:], wv_d[:])
        nu_sb = const.tile([1, 128], bf16, tag="nu")
        nc.sync.dma_start(nu_sb[:], nu_d[:])
        wo_sb = const.tile([128, 8, 1024], bf16, tag="wo")
        nc.sync.dma_start(wo_sb[:], wo_d[:])
        bffT_sb = const.tile([128, 32], f32, tag="bffT")
        nc.sync.dma_start(bffT_sb[:], bffT_d[:])
        if has_rq:
            rq_sb = const.tile([128, 1], f32, tag="rq")
            nc.sync.dma_start(rq_sb[:], rq_d[:])
        if has_bk:
            bk_sb = const.tile([128, 1], f32, tag="bk")
            nc.sync.dma_start(bk_sb[:], bk_d[:])
        if has_bv:
            onesrow_sb = const.tile([1, 128], bf16, tag="onesrow")
            nc.sync.dma_start(onesrow_sb[:], onesrow_d[:])
            bvr_sb = const.tile([1, 128], bf16, tag="bvr")
            nc.sync.dma_start(bvr_sb[:], bvr_d[:])
        if has_bo:
            bo_sb = const.tile([128, 8], f32, tag="bo")
            nc.sync.dma_start(bo_sb[:], bo_d[:])
        if has_b2:
            b2_sb = const.tile([128, 8], f32, tag="b2")
            nc.sync.dma_start(b2_sb[:], b2_d[:])

        # Token ownership interleaves batches: core c owns tokens
        # [128c,128c+128) of batch 0 AND of batch 1. Each batch gets its own
        # AllToAll: A2A#0 fires mid-attention (hidden under batch-1 work) and
        # the batch-0 post half keeps the PE busy through A2A#1's window.
        TH = TSL // 2            # 128 tokens per (core, batch)
        o_inA = dram.tile([NCORES, 128, TH], bf16, tag="o_inA")
        o_outA = dram.tile([NCORES, 128, TH], bf16, tag="o_outA")
        o_inB = dram.tile([NCORES, 128, TH], bf16, tag="o_inB")
        o_outB = dram.tile([NCORES, 128, TH], bf16, tag="o_outB")
        # half of w1 is prefetched during attention (SBUF is tight there);
        # the other half loads at post-scope start once attention pools free
        w1a = wpool.tile([128, 4, 4096], bf16, tag="w1a")

        # ================= ATTENTION SCOPE =================
        with ExitStack() as att:
            resid = att.enter_context(tc.tile_pool(name="resid", bufs=1))
            qload = att.enter_context(tc.tile_pool(name="qload", bufs=4))
            cload = att.enter_context(tc.tile_pool(name="cload", bufs=2))
            probs = att.enter_context(tc.tile_pool(name="probs", bufs=4))
            small = att.enter_context(tc.tile_pool(name="small", bufs=2))
            evac = att.enter_context(tc.tile_pool(name="evac", bufs=3))

            # ---------------- Phase 1: LN1 stats over qT ----------------
            # all stat rows on partition 0 (engine ops need 32-aligned bases)
            stats = resid.tile([1, 3 * T], f32, tag="stats")
            m_f32 = stats[0:1, 0 * T:1 * T]
            var_row = stats[0:1, 1 * T:2 * T]
            tmp_row = stats[0:1, 2 * T:3 * T]
            qT_tiles = {}
            for qt in range(4):
                qs = slice(512 * qt, 512 * (qt + 1))
                tiles = []
                for ks in range(8):
                    qtile = qload.tile([128, 512], bf16, tag=f"qT_{ks}")
                    nc.sync.dma_start(qtile[:], qT_d[128 * ks:128 * (ks + 1), qs])
                    tiles.append(qtile)
                qT_tiles[qt] = tiles
                m_ps = psum_pj.tile([1, 512], f32, tag="proj")
                s_ps = psum_pj.tile([1, 512], f32, tag="proj")
                for ks in range(8):
                    nc.tensor.matmul(m_ps[:], ones_col[:], tiles[ks][:],
                                     start=(ks == 0), stop=(ks == 7))
                for ks in range(8):
                    sq = small.tile([128, 512], bf16, tag="sq")
                    nc.vector.tensor_mul(sq[:], tiles[ks][:], tiles[ks][:])
                    nc.tensor.matmul(s_ps[:], ones_col[:], sq[:],
                                     start=(ks == 0), stop=(ks == 7))
                nc.vector.tensor_scalar_mul(m_f32[0:1, qs], m_ps[:], 1.0 / D)
                nc.vector.tensor_scalar_mul(var_row[0:1, qs], s_ps[:], 1.0 / D)
            # var -= m^2 ; recip on a [1,T] row is lane-serial on the DVE
            # (~13us), so bounce through DRAM into [128,16] to use all lanes,
            # then broadcast-read rstd to 128 partitions. All bounce DMAs ride
            # the ACT hwdge queue, which is idle during this phase.
            m_row = resid.tile([1, T], bf16, tag="m_row")
            nc.vector.tensor_copy(m_row[:], m_f32[:])
            nc.vector.tensor_mul(tmp_row[:], m_f32[:], m_f32[:])
            nc.vector.tensor_sub(var_row[:], var_row[:], tmp_row[:])
            varD = dram.tile([128, 16], f32, tag="varD")
            rstdD = dram.tile([128, 16], f32, tag="rstdD")
            nc.scalar.dma_start(_row_ap(bass, varD[:], T), var_row[:])
            wrk = resid.tile([128, 64], f32, tag="lnwrk")
            vP, sP, rP, tP = (wrk[:, 0:16], wrk[:, 16:32],
                              wrk[:, 32:48], wrk[:, 48:64])
            nc.scalar.dma_start(vP, varD[:])
            nc.vector.tensor_scalar_add(vP, vP, EPS)
            nc.scalar.activation(sP, vP, AF.Sqrt, bias=0.0, scale=1.0)
            nc.vector.reciprocal(rP, sP)
            nc.vector.tensor_mul(tP, rP, rP)
            nc.vector.tensor_mul(tP, tP, vP)
            nc.vector.tensor_scalar(tP, tP, -0.5, 1.5,
                                    op0=ALU.mult, op1=ALU.add)
            nc.vector.tensor_mul(rP, rP, tP)
            nc.scalar.dma_start(rstdD[:], rP)
            rstd_b = resid.tile([128, T], f32, tag="rstd_b")
            nc.scalar.dma_start(
                rstd_b[:], _bcast_ap(bass, _row_ap(bass, rstdD[:], T), 128))

            # ---------------- Phase 1b: K/V projections, both batches ----------------
            # (independent of LN1 -> keeps the PE busy during the stats chain)
            khT_all = []
            v_tiles_all = []
            for b in range(B):
                khT = resid.tile([128, SB], bf16, tag=f"khT_{b}")
                v_tiles = [[None] * 32 for _ in range(HPC)]
                for g in range(8):            # 512-key groups
                    ksl = slice(512 * g, 512 * (g + 1))
                    ctiles = []
                    for ks in range(8):
                        ctile = cload.tile([128, 512], bf16, tag=f"cT_{ks}")
                        nc.sync.dma_start(
                            ctile[:], cT_d[128 * ks:128 * (ks + 1),
                                           SB * b + 512 * g:SB * b + 512 * (g + 1)])
                        ctiles.append(ctile)
                    kh_ps = psum_pj.tile([128, 512], f32, tag="proj")
                    for ks in range(8):
                        nc.tensor.matmul(kh_ps[:], wk_sb[:, ks, :], ctiles[ks][:],
                                         start=(ks == 0), stop=(ks == 7))
                    if has_bk:
                        nc.scalar.activation(khT[:, ksl], kh_ps[:], AF.Copy,
                                             bias=bk_sb[:], scale=1.0)
                    else:
                        nc.vector.tensor_copy(khT[:, ksl], kh_ps[:])
                    for j in range(4):        # 128-key subtiles -> token-major V
                        kt = 4 * g + j
                        v_ps = psum_pj.tile([128, 512], f32, tag="proj")
                        for ks in range(8):
                            nc.tensor.matmul(
                                v_ps[:, 0:128],
                                ctiles[ks][:, 128 * j:128 * (j + 1)], wv_sb[:, ks, :],
                                start=(ks == 0), stop=(ks == 7 and not has_bv))
                        if has_bv:
                            nc.tensor.matmul(v_ps[:, 0:128], onesrow_sb[:], bvr_sb[:],
                                             start=False, stop=True)
                        for hl in range(HPC):
                            vt = resid.tile([128, 65], bf16, tag=f"v_{b}_{hl}_{kt}")
                            nc.vector.tensor_copy(vt[:, 0:64],
                                                  v_ps[:, 64 * hl:64 * (hl + 1)])
                            nc.gpsimd.memset(vt[:, 64:65], 1.0)
                            v_tiles[hl][kt] = vt
                khT_all.append(khT)
                v_tiles_all.append(v_tiles)

            # prefetch first half of w1 while attention runs (sync queue idle)
            nc.sync.dma_start(w1a[:], w1_d[:, 0:4, :])

            # ---------------- Phase 2: q-projection (both heads fused) ----------------
            qhT = resid.tile([128, T], bf16, tag="qhT")
            for qt in range(4):
                qs = slice(512 * qt, 512 * (qt + 1))
                qh_ps = psum_pj.tile([128, 512], f32, tag="proj")
                for ks in range(8):
                    nc.tensor.matmul(qh_ps[:], wq_sb[:, ks, :], qT_tiles[qt][ks][:],
                                     start=(ks == 0), stop=False)
                nc.tensor.matmul(qh_ps[:], nu_sb[:], m_row[0:1, qs],
                                 start=False, stop=True)
                if has_rq:
                    tmp = evac.tile([128, 512], f32, tag="qevac")
                    nc.vector.tensor_mul(tmp[:], qh_ps[:], rstd_b[:, qs])
                    nc.vector.tensor_scalar_add(qhT[:, qs], tmp[:], rq_sb[:])
                else:
                    nc.vector.tensor_mul(qhT[:, qs], qh_ps[:], rstd_b[:, qs])

            # ---------------- Phase 3: attention ----------------
            for b in range(B):
                khT = khT_all[b]
                v_tiles = v_tiles_all[b]
                for hl in range(HPC):
                    hr = slice(64 * hl, 64 * (hl + 1))
                    o_ps = {}
                    for qt in range(2):
                        o_ps[qt] = psum_o.tile([65, 512], f32, tag="o",
                                               name=f"o_ps_{qt}")
                    av_count = [0, 0]
                    n_av = [16, 32]
                    for blk in range(4):
                        for kt8 in range(8):
                            kt = 8 * blk + kt8
                            p0 = 128 * kt8
                            sc = psum_sc.tile([128, 1024], f32, tag="sc")
                            pr = probs.tile([128, 1024], bf16, tag="probs")
                            qt_list = [qt for qt in range(2) if p0 < 512 * qt + 512]
                            for qt in qt_list:
                                q0 = 512 * qt
                                s0 = max(p0 - q0, 0)
                                nc.tensor.matmul(
                                    sc[:, q0 + s0:q0 + 512],
                                    khT[hr, 128 * kt:128 * (kt + 1)],
                                    qhT[hr, S * b + q0 + s0:S * b + q0 + 512],
                                    start=True, stop=True)
                            # valid region is [p0, 1024); triangle on [p0, p0+128)
                            nc.scalar.activation(pr[:, p0:1024], sc[:, p0:1024],
                                                 AF.Exp, bias=negc[:], scale=SCALE)
                            nc.vector.tensor_mul(pr[:, p0:p0 + 128],
                                                 pr[:, p0:p0 + 128], tri[:])
                            for qt in qt_list:
                                q0 = 512 * qt
                                s0 = max(p0 - q0, 0)
                                nc.tensor.matmul(
                                    o_ps[qt][:, s0:512],
                                    v_tiles[hl][kt][:],
                                    pr[:, q0 + s0:q0 + 512],
                                    start=(av_count[qt] == 0),
                                    stop=(av_count[qt] == n_av[qt] - 1))
                                av_count[qt] += 1
                    # normalize + ship shards (token t of this batch -> core t//TH)
                    o_in_b = o_inA if b == 0 else o_inB
                    for qt in range(2):
                        d_row = small.tile([1, 512], f32, tag="d_row")
                        nc.vector.reciprocal(d_row[:], o_ps[qt][64:65, :])
                        db_ps = psum_pj.tile([64, 512], f32, tag="proj")
                        nc.tensor.matmul(db_ps[:], onesf[0:1, 0:64], d_row[:],
                                         start=True, stop=True)
                        d_b = small.tile([64, 512], f32, tag="d_b")
                        nc.scalar.copy(d_b[:], db_ps[:])
                        oT_sb = evac.tile([64, 512], bf16, tag="oT")
                        nc.vector.tensor_mul(oT_sb[:], o_ps[qt][0:64, :], d_b[:])
                        for j in range(4):
                            nc.gpsimd.dma_start(
                                o_in_b[4 * qt + j, 64 * hl:64 * (hl + 1), :],
                                oT_sb[:, 128 * j:128 * (j + 1)])
                # after batch 0 finishes, fire its AllToAll; it completes
                # while batch-1 attention still runs
                if b == 0:
                    if os.environ.get("KERNEL_NO_COLLECTIVE"):
                        nc.gpsimd.dma_start(o_outA[:], o_inA[:])
                    else:
                        nc.gpsimd.collective_compute(
                            "AllToAll", mybir.AluOpType.bypass,
                            replica_groups=[list(range(NCORES))],
                            ins=[o_inA[:].opt()], outs=[o_outA[:].opt()])

        # ---------------- Phase 5: AllToAll for batch 1 ----------------
        if os.environ.get("KERNEL_NO_COLLECTIVE"):
            nc.gpsimd.dma_start(o_outB[:], o_inB[:])
        else:
            nc.gpsimd.collective_compute(
                "AllToAll", mybir.AluOpType.bypass,
                replica_groups=[list(range(NCORES))],
                ins=[o_inB[:].opt()], outs=[o_outB[:].opt()])

        # ================= POST SCOPE (token-parallel, one pass per batch) =====
        with ExitStack() as post:
            resid2 = post.enter_context(tc.tile_pool(name="resid2", bufs=1))
            wstream = post.enter_context(tc.tile_pool(name="wstream", bufs=2))
            small2 = post.enter_context(tc.tile_pool(name="small2", bufs=2))
            evac2 = post.enter_context(tc.tile_pool(name="evac2", bufs=3))

            w1b = resid2.tile([128, 4, 4096], bf16, tag="w1b")
            nc.sync.dma_start(w1b[:], w1_d[:, 4:8, :])

            for half in range(2):
                o_out_h = o_outA if half == 0 else o_outB
                tsl = slice(TH * half, TH * (half + 1))
                o_sb = []
                for ks in range(8):
                    # sync queue, NOT gpsimd: the gpsimd queue blocks on the
                    # second collective's completion wait, which would defeat
                    # the post-half-0 / A2A#1 overlap
                    ot = resid2.tile([128, TH], bf16, tag=f"o_all_{half}_{ks}")
                    nc.sync.dma_start(ot[:], o_out_h[ks, :, :])
                    o_sb.append(ot)
                qsT_sb = []
                for ks in range(8):
                    qst = resid2.tile([128, TH], f32, tag=f"qsT_{half}_{ks}")
                    nc.sync.dma_start(qst[:],
                                      qsT_d[128 * ks:128 * (ks + 1), tsl])
                    qsT_sb.append(qst)
                xT = []
                for dm in range(8):
                    a_ps = psum_pj.tile([128, TH], f32, tag="proj")
                    for ks in range(8):
                        nc.tensor.matmul(a_ps[:],
                                         wo_sb[:, ks, 128 * dm:128 * (dm + 1)],
                                         o_sb[ks][:], start=(ks == 0),
                                         stop=(ks == 7))
                    xt = resid2.tile([128, TH], f32, tag=f"xT_{half}_{dm}")
                    if has_bo:
                        nc.vector.scalar_tensor_tensor(
                            xt[:], a_ps[:], bo_sb[:, dm:dm + 1], qsT_sb[dm][:],
                            op0=ALU.add, op1=ALU.add)
                    else:
                        nc.vector.tensor_add(xt[:], a_ps[:], qsT_sb[dm][:])
                    xT.append(xt)
                    if debug:
                        nc.sync.dma_start(
                            dbgx_d[128 * dm:128 * (dm + 1), tsl], xt[:])
                        nc.gpsimd.dma_start(
                            dbgo_d[128 * dm:128 * (dm + 1), tsl], o_sb[dm][:])

                # LN2 stats
                m2_ps = psum_pj.tile([1, TH], f32, tag="proj")
                s2_ps = psum_pj.tile([1, TH], f32, tag="proj")
                xTb = []
                for dm in range(8):
                    xb = small2.tile([128, TH], bf16, tag=f"xTb_{half}_{dm}",
                                     bufs=1)
                    nc.vector.tensor_copy(xb[:], xT[dm][:])
                    xTb.append(xb)
                for dm in range(8):
                    nc.tensor.matmul(m2_ps[:], ones_col[:], xTb[dm][:],
                                     start=(dm == 0), stop=(dm == 7))
                for dm in range(8):
                    sq2 = small2.tile([128, TH], bf16, tag="sq2")
                    nc.vector.tensor_mul(sq2[:], xTb[dm][:], xTb[dm][:])
                    nc.tensor.matmul(s2_ps[:], ones_col[:], sq2[:],
                                     start=(dm == 0), stop=(dm == 7))
                st2 = resid2.tile([1, 4 * TH], f32, tag=f"st2_{half}")
                m2_row = st2[0:1, 0 * TH:1 * TH]
                var2 = st2[0:1, 1 * TH:2 * TH]
                tmp2 = st2[0:1, 2 * TH:3 * TH]
                rstd2 = st2[0:1, 3 * TH:4 * TH]
                nc.vector.tensor_scalar_mul(m2_row[:], m2_ps[:], 1.0 / D)
                nc.vector.tensor_scalar_mul(var2[:], s2_ps[:], 1.0 / D)
                nc.vector.tensor_mul(tmp2[:], m2_row[:], m2_row[:])
                nc.vector.tensor_sub(var2[:], var2[:], tmp2[:])
                nc.vector.tensor_scalar_add(var2[:], var2[:], EPS)
                nc.scalar.activation(tmp2[:], var2[:], AF.Sqrt,
                                     bias=0.0, scale=1.0)
                nc.vector.reciprocal(rstd2[:], tmp2[:])
                nc.vector.tensor_mul(tmp2[:], rstd2[:], rstd2[:])
                nc.vector.tensor_mul(tmp2[:], tmp2[:], var2[:])
                nc.vector.tensor_scalar(tmp2[:], tmp2[:], -0.5, 1.5,
                                        op0=ALU.mult, op1=ALU.add)
                nc.vector.tensor_mul(rstd2[:], rstd2[:], tmp2[:])
                # broadcast m2/rstd2 to 128 partitions via K=1 fp32 ones-matmuls
                m2_b = resid2.tile([128, TH], f32, tag=f"m2_b_{half}")
                rstd2_b = resid2.tile([128, TH], f32, tag=f"rstd2_b_{half}")
                mb_ps = psum_pj.tile([128, TH], f32, tag="proj")
                nc.tensor.matmul(mb_ps[:], onesf[:], m2_row[:],
                                 start=True, stop=True)
                nc.vector.tensor_copy(m2_b[:], mb_ps[:])
                rb2_ps = psum_pj.tile([128, TH], f32, tag="proj")
                nc.tensor.matmul(rb2_ps[:], onesf[:], rstd2[:],
                                 start=True, stop=True)
                nc.vector.tensor_copy(rstd2_b[:], rb2_ps[:])
                hT = []
                for dm in range(8):
                    ht = resid2.tile([128, TH], bf16, tag=f"hT_{half}_{dm}")
                    tmp = small2.tile([128, TH], f32, tag="hsub")
                    nc.vector.tensor_sub(tmp[:], xT[dm][:], m2_b[:])
                    nc.vector.tensor_mul(ht[:], tmp[:], rstd2_b[:])
                    hT.append(ht)
                    if debug:
                        nc.gpsimd.dma_start(
                            dbgh_d[128 * dm:128 * (dm + 1), tsl], ht[:])

                # ---------------- FFN ----------------
                ff1T = []
                for ft in range(32):
                    f_ps = psum_pj.tile([128, TH], f32, tag="proj")
                    for ks in range(8):
                        w1src = (w1a[:, ks, 128 * ft:128 * (ft + 1)] if ks < 4
                                 else w1b[:, ks - 4, 128 * ft:128 * (ft + 1)])
                        nc.tensor.matmul(f_ps[:], w1src, hT[ks][:],
                                         start=(ks == 0), stop=(ks == 7))
                    f1 = resid2.tile([128, TH], bf16, tag=f"ff1T_{half}_{ft}",
                                     name=f"ff1T_{half}_{ft}")
                    nc.scalar.activation(f1[:], f_ps[:], AF.Gelu,
                                         bias=bffT_sb[:, ft:ft + 1], scale=1.0)
                    ff1T.append(f1)
                    if debug:
                        nc.gpsimd.dma_start(
                            dbgf_d[128 * ft:128 * (ft + 1), tsl], f1[:])
                # one PSUM *bank* per dm accumulation group — interleaved groups
                # sharing a bank corrupt each other via start=True clears
                f2a = psum_sc.tile([128, 1024], f32, tag="sc", name=f"f2a_{half}")
                f2b = psum_sc.tile([128, 1024], f32, tag="sc", name=f"f2b_{half}")
                f2c = psum_pj.tile([128, 512], f32, tag="proj", name=f"f2c_{half}")
                f2d = psum_pj.tile([128, 512], f32, tag="proj", name=f"f2d_{half}")
                f2e = psum_o.tile([128, 512], f32, tag="o", name=f"f2e_{half}")
                f2f = psum_o.tile([128, 512], f32, tag="o", name=f"f2f_{half}")
                f2slots = [f2a[:, 0:TH], f2a[:, 512:512 + TH],
                           f2b[:, 0:TH], f2b[:, 512:512 + TH],
                           f2c[:, 0:TH], f2d[:, 0:TH],
                           f2e[:, 0:TH], f2f[:, 0:TH]]
                for ftp in range(16):
                    w2c = wstream.tile([128, 2, 1024], bf16, tag="w2c")
                    nc.sync.dma_start(w2c[:], w2_d[:, 2 * ftp:2 * (ftp + 1), :])
                    for i in range(2):
                        ft = 2 * ftp + i
                        for dm in range(8):
                            nc.tensor.matmul(
                                f2slots[dm],
                                w2c[:, i, 128 * dm:128 * (dm + 1)], ff1T[ft][:],
                                start=(ft == 0), stop=(ft == 31))
                for dm in range(8):
                    out_sb = evac2.tile([128, TH], f32, tag="out_sb")
                    f2ps = f2slots[dm]
                    if has_b2:
                        nc.vector.scalar_tensor_tensor(
                            out_sb[:], f2ps, b2_sb[:, dm:dm + 1], xT[dm][:],
                            op0=ALU.add, op1=ALU.add)
                    else:
                        nc.vector.tensor_add(out_sb[:], f2ps, xT[dm][:])
                    nc.sync.dma_start(outT_d[128 * dm:128 * (dm + 1), tsl],
                                      out_sb[:])

    _split_excess_waits(nc, mybir)
    _fix_sem_range_clear(nc, mybir)
    return nc


def _fix_sem_range_clear(nc, mybir):
    """The installed walrus rejects bass's 64-byte EVENT_SEMAPHORE_RANGE_CLEAR
    encoding ("ISA wrong length"); it expects the 16-byte sequencer form.
    All payload lives in the first 16 bytes, so truncate."""
    k = 0
    for f in nc.m.functions:
        for bb in f.blocks:
            out = []
            changed = False
            for ins in bb.instructions:
                if (type(ins).__name__ == "InstISA"
                        and ins.op_name == "EVENT_SEMAPHORE_RANGE_CLEAR"):
                    changed = True
                    d = ins.ant_dict
                    si = getattr(ins, "sync_info", None)
                    waits = list(si.on_wait) if si else []
                    upds = list(si.on_update) if si else []
                    sems = list(range(d["range_first"], d["range_last"] + 1))
                    for i, s in enumerate(sems):
                        es = mybir.InstEventSemaphore(
                            name=f"I-semclr-{k}", ins=[], outs=[])
                        k += 1
                        es.engine = ins.engine
                        u = [mybir.SyncUpdate(sync_type="semaphore", id=s,
                                              ant_name=f"semclr_{s}",
                                              update_mode="sem-wr-imm",
                                              update_value=0)]
                        if i == len(sems) - 1:
                            u += upds
                        es.sync_info = mybir.SyncInfo(
                            on_wait=(waits if i == 0 else []), on_update=u)
                        out.append(es)
                    continue
                out.append(ins)
            if changed:
                bb.instructions = out


_SPLIT_TYPES = {
    "InstMatmult", "InstTensorTensor", "InstActivation", "InstTensorCopy",
    "InstTensorScalar", "InstTensorScalarPtr", "InstCustomDveAnt",
    "InstMemset", "InstReciprocal", "InstTensorReduce", "InstLdWeights",
    "InstLoadStationary", "InstNoOp", "InstTranspose", "InstScalarTensorTensor",
    "InstDMACopy", "InstLdweights", "InstCollectiveCompute", "InstDrain",
}


def _split_excess_waits(nc, mybir, max_waits=1):
    """Compute-engine instructions support only `max_waits` sync waits; Tile
    sometimes emits more. Hoist the excess onto same-engine NoOps inserted
    immediately before (engines run in order, so this is semantics-preserving)."""
    fix = 0
    for f in nc.m.functions:
        for bb in f.blocks:
            out = []
            changed = False
            for ins in bb.instructions:
                si = getattr(ins, "sync_info", None)
                if (si is not None and len(si.on_wait) > max_waits
                        and type(ins).__name__ in _SPLIT_TYPES):
                    waits = list(si.on_wait)
                    keep = waits[-max_waits:]
                    excess = waits[:-max_waits]
                    while excess:
                        chunk, excess = excess[:max_waits], excess[max_waits:]
                        nop = mybir.InstEventSemaphore(
                            name=f"I-waitfix-{fix}", ins=[], outs=[])
                        fix += 1
                        nop.engine = ins.engine
                        nop.sync_info = mybir.SyncInfo(on_wait=chunk, on_update=[])
                        out.append(nop)
                    ins.sync_info = mybir.SyncInfo(on_wait=keep,
                                                   on_update=list(si.on_update))
                    changed = True
                out.append(ins)
            if changed:
                bb.instructions = out


def _host_prep(inputs):
    import ml_dtypes
    BF = ml_dtypes.bfloat16
    I = {k: np.ascontiguousarray(np.asarray(v, np.float32)) for k, v in inputs.items()}

    qf = I['query'].reshape(T, D)
    cf = I['context'].reshape(TC, D)
    qT = np.ascontiguousarray(qf.T).astype(BF)
    cT = np.ascontiguousarray(cf.T).astype(BF)

    wq_eff = I['ln1_g'][:, None] * I['wq']
    rq = I['ln1_b'] @ I['wq'] + I['bq']            # (1024,)
    w1_eff = I['ln2_g'][:, None] * I['w1']
    bff = I['b1'] + I['ln2_b'] @ I['w1']           # (4096,)

    def wtile(w, nk):   # (nk*128, m) -> (128, nk, m)
        m = w.shape[1]
        return np.ascontiguousarray(
            w.reshape(nk, 128, m).transpose(1, 0, 2)).astype(BF)

    common = {
        "qT": qT, "cT": cT,
        "wo": wtile(I['wo'], 8),
        "w1": wtile(w1_eff, 8),
        "w2": wtile(I['w2'], 32),
        "bffT": np.ascontiguousarray(bff.reshape(32, 128).T).astype(np.float32),
        "tri": np.triu(np.ones((128, 128), np.float32)).astype(BF),
        "ones_bf": np.ones((128, 1), np.float32).astype(BF),
        "ones_f32": np.ones((1, 128), np.float32),
    }

    flags = (bool(np.any(rq != 0)), bool(np.any(I['bk'] != 0)),
             bool(np.any(I['bv'] != 0)), bool(np.any(I['bo'] != 0)),
             bool(np.any(I['b2'] != 0)))

    in_maps = []
    for c in range(NCORES):
        cols = slice(128 * c, 128 * (c + 1))
        wq_c = wq_eff[:, cols].astype(BF).astype(np.float32)
        m = dict(common)
        # core c owns tokens [128c,128c+128) of batch 0 and of batch 1
        th = TSL // 2
        m["qsT"] = np.ascontiguousarray(np.concatenate(
            [qf.T[:, th * c:th * (c + 1)],
             qf.T[:, S + th * c:S + th * (c + 1)]], axis=1))
        m["wq"] = wtile(wq_eff[:, cols], 8)
        m["nu"] = (-wq_c.sum(axis=0, keepdims=True)).astype(BF)
        m["wk"] = wtile(I['wk'][:, cols], 8)
        m["wv"] = wtile(I['wv'][:, cols], 8)
        if flags[0]:
            m["rqT"] = rq[cols].reshape(128, 1).astype(np.float32)
        if flags[1]:
            m["bkT"] = I['bk'][cols].reshape(128, 1).astype(np.float32)
        if flags[2]:
            m["ones_row"] = np.ones((1, 128), np.float32).astype(BF)
            m["bvr"] = I['bv'][cols].reshape(1, 128).astype(BF)
        if flags[3]:
            m["boT"] = np.ascontiguousarray(
                I['bo'].reshape(8, 128).T).astype(np.float32)
        if flags[4]:
            m["b2T"] = np.ascontiguousarray(
                I['b2'].reshape(8, 128).T).astype(np.float32)
        in_maps.append(m)
    return in_maps, flags


def kernel(**inputs):
    _, _, _, run_bass_kernel_spmd = _import_concourse()
    in_maps, flags = _host_prep(inputs)
    if flags not in _CACHE:
        _CACHE[flags] = _build_program(flags)
    nc = _CACHE[flags]
    res = run_bass_kernel_spmd(nc, in_maps, core_ids=list(range(NCORES)))
    th = TSL // 2
    outT = np.empty((D, T), np.float32)
    for c in range(NCORES):
        oc = np.asarray(res.results[c]["outT"], np.float32)   # (1024, 256)
        outT[:, th * c:th * (c + 1)] = oc[:, 0:th]            # batch 0 tokens
        outT[:, S + th * c:S + th * (c + 1)] = oc[:, th:]     # batch 1 tokens
    return np.ascontiguousarray(outT.T).reshape(B, S, D).astype(np.float32)


if __name__ == "__main__":
    expected = np.load('/root/problem/expected.npy')
    data = np.load('/root/problem/inputs.npz')
    act = kernel(**{k: data[k] for k in data.files})
    rel = np.linalg.norm(act - expected) / np.linalg.norm(expected)
    print("Relative error:", rel)

